# revision 9
# baseline (speedup 1.0000x reference)
"""Trainium2 Bass kernel for KeypointSampler: 8x8-window gumbel-max sampling.

Sharding: pure data-parallel over batch B=16 -> 8 cores x 2 samples.
Device layout: partition = cell-row (cy), free = (r, col) image rows.
"""
import functools
import os

import numpy as np

B, H, W = 16, 2048, 2048
Hc, Wc, WIN = 256, 256, 8
NCORES = 8
BPC = B // NCORES  # samples per core
NH = 2             # cy halves per sample (128 cell-rows each)
NJ = 8             # column blocks per half (256 image cols = 32 cells)
CJ = 32            # cells per block along x
COLS = CJ * WIN    # 256 image columns per block

_rng_cache = None
last_results = None  # BassKernelResults of the most recent run (for test.py)


def _get_rng():
    """Reproduce the reference's jax randomness (key 42) exactly, on CPU."""
    global _rng_cache
    if _rng_cache is None:
        import jax
        import jax.numpy as jnp

        cpu = jax.devices("cpu")[0]
        with jax.default_device(cpu):
            key = jax.random.key(42)
            kg, kb = jax.random.split(key)
            g = jax.random.gumbel(kg, (B, 1, Hc, Wc, WIN * WIN), dtype=jnp.float32)
            u = jax.random.uniform(kb, (B, 1, Hc, Wc), dtype=jnp.float32)
        g = np.asarray(g)
        u = np.asarray(u)
        # inverse-gridify gumbel to image layout [B, H, W]
        g_img = np.ascontiguousarray(
            g.reshape(B, Hc, Wc, WIN, WIN)
            .transpose(0, 1, 3, 2, 4)
            .reshape(B, H, W)
        )
        u = np.ascontiguousarray(u.reshape(B, Hc, Wc))
        _rng_cache = (g_img, u)
    return _rng_cache


def _ap(base, offset_delta, dims):
    """Raw AP on the same tensor as `base` (an AP), with explicit [step,count] dims."""
    import concourse.bass as bass

    return bass.AP(tensor=base.tensor, offset=base.offset + offset_delta, ap=list(dims))


@functools.lru_cache(maxsize=1)
def _build():
    import concourse.bass as bass
    import concourse.bacc as bacc
    import concourse.tile as tile
    from concourse import mybir

    f32 = mybir.dt.float32
    bf16 = mybir.dt.bfloat16
    i32 = mybir.dt.int32
    u8 = mybir.dt.uint8
    Alu = mybir.AluOpType
    Act = mybir.ActivationFunctionType
    AX = mybir.AxisListType

    nc = bacc.Bacc("TRN2", debug=False, enable_asserts=False, num_devices=NCORES)

    x_d = nc.dram_tensor("x", [BPC, H, W], f32, kind="ExternalInput").ap()
    g_d = nc.dram_tensor("g", [BPC, H, W], f32, kind="ExternalInput").ap()
    mp_d = nc.dram_tensor("mp", [BPC, H, W], f32, kind="ExternalInput").ap()
    u_d = nc.dram_tensor("u", [BPC, Hc, Wc], f32, kind="ExternalInput").ap()
    kp_d = nc.dram_tensor("kp", [BPC, Hc, Wc, 2], f32, kind="ExternalOutput").ap()
    lp_d = nc.dram_tensor("lp", [BPC, Hc, Wc], f32, kind="ExternalOutput").ap()
    mk_d = nc.dram_tensor("mk", [BPC, Hc, Wc], u8, kind="ExternalOutput").ap()
    mg_d = nc.dram_tensor("mg", [BPC, Hc, Wc], f32, kind="ExternalOutput").ap()
    ls_d = nc.dram_tensor("ls", [BPC, Hc, Wc], f32, kind="ExternalOutput").ap()

    FD = WIN * COLS  # free elems per partition per block tile

    import contextlib

    with tile.TileContext(nc) as tc, contextlib.ExitStack() as ctx:
        consts = ctx.enter_context(tc.tile_pool(name="consts", bufs=1))
        pblk = ctx.enter_context(tc.tile_pool(name="pblk", bufs=2))
        phalf = ctx.enter_context(tc.tile_pool(name="phalf", bufs=2))
        ptail = ctx.enter_context(tc.tile_pool(name="ptail", bufs=1))

        # ---- one-time constants ----
        # C2[r, col] = 8*r + (col % 8), bf16 (exact: ints < 256)
        c2_i = consts.tile([128, WIN, COLS], i32)
        nc.gpsimd.iota(
            _ap(c2_i[:], 0, [c2_i[:].ap[0], [COLS, WIN], [8, CJ], [1, WIN]]),
            pattern=[[WIN, WIN], [0, CJ], [1, WIN]],
            channel_multiplier=0,
        )
        c2 = consts.tile([128, WIN, COLS], bf16)
        nc.vector.tensor_copy(c2[:], c2_i[:])
        # CX8[p, cell] = 8*cell  (f32, cell in 0..255)
        cx8_i = consts.tile([128, Wc], i32)
        nc.gpsimd.iota(cx8_i[:], pattern=[[WIN, Wc]], channel_multiplier=0)
        cx8 = consts.tile([128, Wc], f32)
        nc.vector.tensor_copy(cx8[:], cx8_i[:])
        # per-partition 8*(128*h + p) for each half
        cyb = []
        for hh in range(NH):
            t_i = consts.tile([128, 1], i32, tag=f"cyb{hh}i")
            nc.gpsimd.iota(t_i[:], pattern=[[0, 1]], base=WIN * 128 * hh,
                           channel_multiplier=WIN)
            t_f = consts.tile([128, 1], f32, tag=f"cyb{hh}f")
            nc.vector.tensor_copy(t_f[:], t_i[:])
            cyb.append(t_f)

        def cells_view(t, dims_extra=None):
            """[128, WIN, COLS] tile -> [p, cx(CJ), r(WIN), cc(WIN)] view."""
            a = t[:]
            return _ap(a, 0, [a.ap[0], [WIN, CJ], [COLS, WIN], [1, WIN]])

        for b in range(BPC):
            for hh in range(NH):
                A_h = phalf.tile([128, Wc], bf16, tag="Ah")
                v_h = phalf.tile([128, Wc], f32, tag="vh")
                S_h = phalf.tile([128, Wc], f32, tag="Sh")
                m_h = phalf.tile([128, Wc], f32, tag="mh")
                u_t = phalf.tile([128, Wc], f32, tag="uh")
                nc.sync.dma_start(
                    out=u_t[:],
                    in_=_ap(u_d, b * Hc * Wc + hh * 128 * Wc,
                            [[Wc, 128], [1, Wc]]),
                )

                for j in range(NJ):
                    img_off = b * H * W + hh * 128 * WIN * W + j * COLS
                    src_dims = [[WIN * W, 128], [W, WIN], [1, COLS]]

                    x_t = pblk.tile([128, WIN, COLS], f32, tag="x")
                    g_t = pblk.tile([128, WIN, COLS], f32, tag="g")
                    mp_t = pblk.tile([128, WIN, COLS], f32, tag="mp")
                    nc.sync.dma_start(out=x_t[:], in_=_ap(x_d, img_off, src_dims))
                    nc.sync.dma_start(out=g_t[:], in_=_ap(g_d, img_off, src_dims))
                    nc.sync.dma_start(out=mp_t[:], in_=_ap(mp_d, img_off, src_dims))

                    s_t = pblk.tile([128, WIN, COLS], f32, tag="s")
                    nc.vector.tensor_tensor(s_t[:], x_t[:], g_t[:], op=Alu.add)

                    M_t = pblk.tile([128, CJ], f32, tag="M")
                    nc.vector.tensor_reduce(M_t[:], cells_view(s_t), axis=AX.XY,
                                            op=Alu.max)
                    Mb = _ap(M_t[:], 0,
                             [M_t[:].ap[0], [1, CJ], [0, WIN], [0, WIN]])

                    eq_t = pblk.tile([128, WIN, COLS], u8, tag="eq")
                    nc.vector.tensor_tensor(cells_view(eq_t), cells_view(s_t), Mb,
                                            op=Alu.is_ge)

                    T2_t = pblk.tile([128, WIN, COLS], bf16, tag="T2")
                    nc.vector.scalar_tensor_tensor(
                        T2_t[:], eq_t[:], -64.0, c2[:],
                        op0=Alu.mult, op1=Alu.add)
                    nc.vector.tensor_reduce(
                        A_h[:, j * CJ:(j + 1) * CJ], cells_view(T2_t),
                        axis=AX.XY, op=Alu.min)

                    # value at argmax: x*eq is x[c] at the argmax, 0 elsewhere;
                    # summing the window recovers x[c] exactly (no ties).
                    xe_t = pblk.tile([128, WIN, COLS], f32, tag="xe")
                    nc.vector.tensor_tensor(xe_t[:], x_t[:], eq_t[:], op=Alu.mult)
                    nc.vector.tensor_reduce(
                        v_h[:, j * CJ:(j + 1) * CJ], cells_view(xe_t),
                        axis=AX.XY, op=Alu.add)

                    e_t = pblk.tile([128, WIN, COLS], f32, tag="e")
                    nc.scalar.activation(e_t[:], x_t[:], Act.Exp)
                    nc.vector.tensor_reduce(
                        S_h[:, j * CJ:(j + 1) * CJ], cells_view(e_t),
                        axis=AX.XY, op=Alu.add)

                    nc.vector.tensor_reduce(
                        m_h[:, j * CJ:(j + 1) * CJ], cells_view(mp_t),
                        axis=AX.XY, op=Alu.min)

                # ---- per-half tail on [128, 256] cell tiles ----
                out_off = b * Hc * Wc + hh * 128 * Wc
                idx_f = ptail.tile([128, Wc], f32, tag="idx")
                nc.vector.tensor_scalar_add(idx_f[:], A_h[:], 64.0)
                ci = ptail.tile([128, Wc], i32, tag="ci")
                nc.vector.tensor_copy(ci[:], idx_f[:])
                ri = ptail.tile([128, Wc], i32, tag="ri")
                nc.vector.tensor_scalar(ri[:], ci[:], 3, None,
                                        op0=Alu.logical_shift_right)
                cci = ptail.tile([128, Wc], i32, tag="cci")
                nc.vector.tensor_scalar(cci[:], ci[:], 7, None,
                                        op0=Alu.bitwise_and)
                rf = ptail.tile([128, Wc], f32, tag="rf")
                nc.vector.tensor_copy(rf[:], ri[:])
                ccf = ptail.tile([128, Wc], f32, tag="ccf")
                nc.vector.tensor_copy(ccf[:], cci[:])

                kxy = ptail.tile([128, Wc, 2], f32, tag="kxy")
                kx_view = _ap(kxy[:], 0, [kxy[:].ap[0], [2, Wc]])
                ky_view = _ap(kxy[:], 1, [kxy[:].ap[0], [2, Wc]])
                nc.vector.tensor_tensor(kx_view, ccf[:], cx8[:], op=Alu.add)
                nc.vector.tensor_scalar_add(ky_view, rf[:], cyb[hh][:])

                lse = ptail.tile([128, Wc], f32, tag="lse")
                nc.scalar.activation(lse[:], S_h[:], Act.Ln)
                sig = ptail.tile([128, Wc], f32, tag="sig")
                nc.scalar.activation(sig[:], v_h[:], Act.Sigmoid)
                acc = ptail.tile([128, Wc], f32, tag="acc")
                nc.vector.tensor_tensor(acc[:], u_t[:], sig[:], op=Alu.is_lt)
                sgm = ptail.tile([128, Wc], f32, tag="sgm")
                nc.scalar.activation(sgm[:], v_h[:], Act.Sigmoid, scale=-1.0)
                lsg = ptail.tile([128, Wc], f32, tag="lsg")
                nc.scalar.activation(lsg[:], sgm[:], Act.Ln)
                t1 = ptail.tile([128, Wc], f32, tag="t1")
                nc.vector.tensor_tensor(t1[:], v_h[:], lse[:], op=Alu.subtract)
                t2 = ptail.tile([128, Wc], f32, tag="t2")
                nc.vector.tensor_tensor(t2[:], t1[:], lsg[:], op=Alu.add)
                av = ptail.tile([128, Wc], f32, tag="av")
                nc.vector.tensor_tensor(av[:], acc[:], v_h[:], op=Alu.mult)
                lp_t = ptail.tile([128, Wc], f32, tag="lp")
                nc.vector.tensor_tensor(lp_t[:], t2[:], av[:], op=Alu.add)
                mk_t = ptail.tile([128, Wc], u8, tag="mk")
                nc.vector.tensor_copy(mk_t[:], acc[:])

                nc.sync.dma_start(
                    out=_ap(kp_d, out_off * 2, [[Wc * 2, 128], [2, Wc], [1, 2]]),
                    in_=kxy[:])
                odims = [[Wc, 128], [1, Wc]]
                nc.sync.dma_start(out=_ap(lp_d, out_off, odims), in_=lp_t[:])
                nc.sync.dma_start(out=_ap(mk_d, out_off, odims), in_=mk_t[:])
                nc.sync.dma_start(out=_ap(mg_d, out_off, odims), in_=m_h[:])
                nc.sync.dma_start(out=_ap(ls_d, out_off, odims), in_=v_h[:])

    nc.compile()
    return nc


def kernel(x, mask_padding):
    global last_results
    from concourse import bass_utils

    x = np.ascontiguousarray(np.asarray(x, dtype=np.float32))
    mp = np.ascontiguousarray(np.asarray(mask_padding, dtype=np.float32))
    g_img, u = _get_rng()

    nc = _build()
    in_maps = []
    for c in range(NCORES):
        s = slice(c * BPC, (c + 1) * BPC)
        in_maps.append({
            "x": np.ascontiguousarray(x[s, 0]),
            "g": np.ascontiguousarray(g_img[s]),
            "mp": np.ascontiguousarray(mp[s, 0]),
            "u": np.ascontiguousarray(u[s]),
        })

    res = bass_utils.run_bass_kernel_spmd(
        nc, in_maps, core_ids=list(range(NCORES)),
        trace=bool(int(os.environ.get("KP_TRACE", "0"))),
    )
    last_results = res
    outs = res.results

    kp = np.concatenate([r["kp"] for r in outs], axis=0)
    lp = np.concatenate([r["lp"] for r in outs], axis=0)
    mk = np.concatenate([r["mk"] for r in outs], axis=0).astype(bool)
    mg = np.concatenate([r["mg"] for r in outs], axis=0).reshape(B, 1, Hc, Wc)
    ls = np.concatenate([r["ls"] for r in outs], axis=0)
    return kp, lp, mk, mg, ls
